# revision 1
# baseline (speedup 1.0000x reference)
"""Multi-head causal attention with RoPE on 8 Trainium2 NeuronCores.

Problem: B=2, S=2048, D=1024, H=16 heads (dk=64), fp32, causal mask,
RoPE on Q/K, y = softmax(QK^T/sqrt(dk)) V projected by Wo.

Sharding: head-parallel. Core c owns 2 heads (columns c*128:(c+1)*128 of
the QKV projection output). Each core:
  1. computes Q^T,K^T,V for its heads from the full x (K-dim 1024 matmuls),
  2. applies RoPE in the transposed [head_dim, token] layout,
  3. runs causal attention with scores materialized transposed (ST[k,q]) so
     softmax needs no transposes: exp on ScalarE straight out of PSUM, the
     PV matmul consumes the exp'd tile as the moving operand, and a ones
     column appended to V makes the same matmul emit the softmax denominator,
  4. AllToAll (2 MB) flips head-sharded -> token-sharded,
  5. computes 1/8 of the output projection; host concatenates row slices.
A tiny dummy AllToAll at kernel start absorbs the one-time collective
warmup cost under the projection phase.
"""

import sys

for p in ("/opt/trn_rl_repo", "/root/.axon_site/_ro/trn_rl_repo"):
    if p not in sys.path:
        sys.path.insert(0, p)

import math

import numpy as np

import concourse.bass as bass
import concourse.tile as tile
from concourse import mybir
from concourse.bass_utils import run_bass_kernel_spmd

N_CORES = 8
B, S, D, H = 2, 2048, 1024, 16
DK = D // H          # 64
HPC = H // N_CORES   # heads per core = 2
FW = HPC * DK        # head-group width per core = 128
T = B * S            # 4096 flattened tokens
TCH = 512            # token chunk for projections
NCH = T // TCH       # 8 chunks
KT = 128             # k tile
QC = 512             # q chunk in attention
TSL = T // N_CORES   # 512 output rows per core

F32 = mybir.dt.float32
F32R = mybir.dt.float32r


def _spill_waits(nc, max_other=1):
    """walrus in this container allows 1 sync-wait per instruction; move
    excess waits onto preceding single-wait NoOps on the same engine."""
    n_new = 0
    for bb in nc.m.functions[0].blocks:
        newlist = []
        changed = False
        for inst in bb.instructions:
            si = inst.sync_info
            if si is not None and si.on_wait and len(si.on_wait) > max_other:
                waits = list(si.on_wait)
                overflow, keep = waits[:-max_other], waits[-max_other:]
                while overflow:
                    chunk, overflow = overflow[:1], overflow[1:]
                    nop = mybir.InstNoOp(
                        name=f"waitspill{n_new}-{inst.name}", ins=[], outs=[]
                    )
                    nop.engine = inst.engine
                    nop.debug = inst.debug
                    nop.sync_info = mybir.SyncInfo(on_wait=chunk, on_update=[])
                    newlist.append(nop)
                    n_new += 1
                si.on_wait = keep
                inst.sync_info = si
                changed = True
            newlist.append(inst)
        if changed:
            bb.instructions = newlist
    return n_new


def build_kernel():
    nc = bass.Bass("TRN2", num_devices=N_CORES)

    xT = nc.dram_tensor("xT", [D, T], F32R, kind="ExternalInput")
    wq = nc.dram_tensor("wq", [D, FW], F32R, kind="ExternalInput")  # pre-scaled 1/sqrt(dk)
    wk = nc.dram_tensor("wk", [D, FW], F32R, kind="ExternalInput")
    wv = nc.dram_tensor("wv", [D, FW], F32R, kind="ExternalInput")
    woT = nc.dram_tensor("woT", [D, D], F32R, kind="ExternalInput")
    ctab = nc.dram_tensor("ctab", [FW, S], F32, kind="ExternalInput")
    stab = nc.dram_tensor("stab", [FW, S], F32, kind="ExternalInput")
    masks = nc.dram_tensor("masks", [KT, 896], F32R, kind="ExternalInput")
    y = nc.dram_tensor("y", [TSL, D], F32, kind="ExternalOutput")

    xT_r = xT.rearrange("(dt p) t -> p dt t", p=128)  # [128, 8, T]

    with tile.TileContext(nc) as tc:
        with (
            tc.tile_pool(name="const", bufs=1) as const,
            tc.tile_pool(name="xch", bufs=2) as xch,
            tc.tile_pool(name="qk", bufs=1) as qkpool,
            tc.tile_pool(name="tmp", bufs=3) as tmp,
            tc.tile_pool(name="pts", bufs=6) as pts,
            tc.tile_pool(name="lpool", bufs=2) as lpool,
            tc.tile_pool(name="wo", bufs=8) as wopool,
            tc.tile_pool(name="yout", bufs=2) as ypool,
            tc.tile_pool(name="mm", bufs=2, space="PSUM") as mmps,
            tc.tile_pool(name="st", bufs=4, space="PSUM") as stps,
            tc.tile_pool(name="pv", bufs=2, space="PSUM") as pvps,
            tc.tile_pool(name="dram", bufs=1, space="DRAM") as dram,
        ):
            # ---- collective warmup (hidden under projection phase) ----
            warm_in = dram.tile([8, 16], F32)
            warm_out = dram.tile([8, 16], F32)
            wtile = const.tile([1, 128], F32)
            nc.vector.memset(wtile, 0.0)
            nc.gpsimd.dma_start(out=warm_in[:, :], in_=wtile[:1, :128].rearrange("p (a f) -> (p a) f", a=8))
            nc.gpsimd.collective_compute(
                "AllToAll",
                mybir.AluOpType.bypass,
                replica_groups=[list(range(N_CORES))],
                ins=[warm_in[:].opt()],
                outs=[warm_out[:].opt()],
            )

            # ---- constants ----
            wq_sb = const.tile([128, 8, FW], F32R)
            wk_sb = const.tile([128, 8, FW], F32R)
            wv_sb = const.tile([128, 8, FW], F32R)
            nc.sync.dma_start(out=wq_sb, in_=wq.rearrange("(dt p) f -> p dt f", p=128))
            nc.sync.dma_start(out=wk_sb, in_=wk.rearrange("(dt p) f -> p dt f", p=128))
            nc.sync.dma_start(out=wv_sb, in_=wv.rearrange("(dt p) f -> p dt f", p=128))
            c_sb = const.tile([FW, S], F32)
            s_sb = const.tile([FW, S], F32)
            nc.sync.dma_start(out=c_sb, in_=ctab[:, :])
            nc.sync.dma_start(out=s_sb, in_=stab[:, :])
            mask_sb = const.tile([KT, 896], F32R)
            nc.sync.dma_start(out=mask_sb, in_=masks[:, :])
            ones_f = const.tile([128, DK], F32)
            nc.vector.memset(ones_f, 1.0)
            ones64 = const.tile([1, DK], F32R)
            nc.vector.tensor_copy(out=ones64, in_=ones_f[:1, :])

            qT = qkpool.tile([FW, T], F32R, tag="qT")
            kTt = qkpool.tile([FW, T], F32R, tag="kT")
            v_sb = qkpool.tile([128, T // 128, 2 * DK + 2], F32R, tag="v")
            outT = qkpool.tile([FW, T], F32R, tag="outT")
            # bake the ones columns (f32r tiles can't be memset directly)
            vones = const.tile([128, T // 128], F32)
            nc.vector.memset(vones, 1.0)
            nc.vector.tensor_copy(out=v_sb[:, :, DK], in_=vones)
            nc.vector.tensor_copy(out=v_sb[:, :, 2 * DK + 1], in_=vones)

            # ---- QKV projections + RoPE (per x chunk) ----
            def do_qkv_chunk(ci):
                t0 = ci * TCH
                sc = (ci % (S // TCH)) * TCH  # position within batch for rope tables
                xc = xch.tile([128, 8, TCH], F32R, tag="x", name="xc")
                nc.sync.dma_start(out=xc, in_=xT_r[:, :, t0 : t0 + TCH])

                # Q accumulates in the "mm" pool, K in the "st" pool so the two
                # groups never stall on the same PSUM slot rotation.
                for which, w_sb, dst, pool, tag in (
                    ("q", wq_sb, qT, mmps, "mm"),
                    ("k", wk_sb, kTt, stps, "st"),
                ):
                    ps = pool.tile([FW, TCH], F32, tag=tag, name=f"{which}ps")
                    for dt in range(8):
                        nc.tensor.matmul(
                            ps,
                            w_sb[:, dt, :],
                            xc[:, dt, :],
                            start=(dt == 0),
                            stop=(dt == 7),
                        )
                    raw = tmp.tile([FW, TCH], F32R, tag="raw", name="raw")
                    nc.vector.tensor_copy(out=raw, in_=ps)
                    swp = tmp.tile([FW, TCH], F32R, tag="swp", name="swp")
                    # pair swap across partitions via two strided DMAs
                    nc.sync.dma_start(out=swp[0 : FW - 1 : 2, :], in_=raw[1:FW:2, :])
                    nc.sync.dma_start(out=swp[1:FW:2, :], in_=raw[0 : FW - 1 : 2, :])
                    dslice = dst[:, t0 : t0 + TCH]
                    nc.vector.tensor_mul(dslice, raw, c_sb[:, sc : sc + TCH])
                    t2 = tmp.tile([FW, TCH], F32R, tag="ropetmp", name="t2")
                    nc.gpsimd.tensor_mul(t2, swp, s_sb[:, sc : sc + TCH])
                    nc.vector.tensor_add(dslice, dslice, t2)

                # V: [token, feature] layout, stationary = x chunk subtiles
                for sub in range(TCH // 128):
                    pool, tag = ((mmps, "mm"), (stps, "st"))[sub % 2]
                    vps = pool.tile([128, 128], F32, tag=tag, name="vps")
                    for dt in range(8):
                        nc.tensor.matmul(
                            vps,
                            xc[:, dt, sub * 128 : (sub + 1) * 128],
                            wv_sb[:, dt, :],
                            start=(dt == 0),
                            stop=(dt == 7),
                        )
                    idx = t0 // 128 + sub
                    # ScalarE (idle during this phase) does the PSUM->SBUF copies
                    nc.scalar.copy(out=v_sb[:, idx, 0:DK], in_=vps[:, 0:DK])
                    nc.scalar.copy(
                        out=v_sb[:, idx, DK + 1 : 2 * DK + 1], in_=vps[:, DK : 2 * DK]
                    )

            # ---- causal attention for one batch ----
            # Transposed-scores flash style. The two heads' score matmuls run
            # concurrently in the PE array (row groups 0-63 / 64-127), and the
            # loop is software-pipelined one k-tile ahead: scores for kt+1 are
            # issued before the PV matmuls of kt, so the PE never waits on exp.
            def emit_st(b, qc, kt):
                trow = b * S + qc * QC
                kcol = b * S + kt * KT
                ptpair = []
                for h2 in range(HPC):
                    fb = h2 * DK
                    st = stps.tile([KT, QC], F32, tag="st", name=f"st{h2}")
                    nc.tensor.matmul(
                        st,
                        kTt[fb : fb + DK, kcol : kcol + KT],
                        qT[fb : fb + DK, trow : trow + QC],
                        start=True,
                        stop=True,
                    )
                    pt = pts.tile([KT, QC], F32R, tag="pt", name=f"pt{h2}")
                    nc.scalar.activation(
                        out=pt, in_=st, func=mybir.ActivationFunctionType.Exp
                    )
                    if kt >= 4 * qc:
                        o = (kt - 4 * qc) * KT
                        nc.gpsimd.tensor_mul(
                            pt, pt, mask_sb[:, 384 - o : 384 - o + QC]
                        )
                    ptpair.append(pt)
                return ptpair

            def emit_pv(b, qc, kt, pv2, ptpair):
                nkt = 4 * (qc + 1)
                for h2 in range(HPC):
                    vcol = h2 * (DK + 1)
                    nc.tensor.matmul(
                        pv2[h2],
                        v_sb[:, b * (S // 128) + kt, vcol : vcol + DK + 1],
                        ptpair[h2],
                        start=(kt == 0),
                        stop=(kt == nkt - 1),
                        skip_group_check=True,
                    )

            def do_attn_batch(b):
                for qc in range(S // QC):
                    trow = b * S + qc * QC
                    pv2 = [
                        pvps.tile([DK + 1, QC], F32, tag="pv", name=f"pv{h2}")
                        for h2 in range(HPC)
                    ]
                    nkt = 4 * (qc + 1)
                    prev = emit_st(b, qc, 0)
                    for kt in range(1, nkt):
                        cur = emit_st(b, qc, kt)
                        emit_pv(b, qc, kt - 1, pv2, prev)
                        prev = cur
                    emit_pv(b, qc, nkt - 1, pv2, prev)
                    for h2 in range(HPC):
                        fb = h2 * DK
                        pv = pv2[h2]
                        linv = lpool.tile([1, QC], F32R, tag="linv", name="linv")
                        with nc.allow_low_precision(reason="f32r rounding of 1/l"):
                            nc.vector.reciprocal(out=linv, in_=pv[DK : DK + 1, :])
                        # broadcast 1/l across the 64 head-dim partitions via a
                        # K=1 ones matmul (engines can't partition-broadcast)
                        lbps = stps.tile([DK, QC], F32, tag="st", name="lbps")
                        nc.tensor.matmul(lbps, ones64, linv, start=True, stop=True)
                        lb = lpool.tile([DK, QC], F32, tag="lb", name="lb")
                        nc.vector.tensor_copy(out=lb, in_=lbps)
                        nc.vector.tensor_mul(
                            outT[fb : fb + DK, trow : trow + QC], pv[0:DK, :], lb
                        )

            # batch 0 projections -> batch 0 attention (hides batch 1's x DMA)
            # -> batch 1 projections -> batch 1 attention
            for ci in range(NCH // 2):
                do_qkv_chunk(ci)
            do_attn_batch(0)
            for ci in range(NCH // 2, NCH):
                do_qkv_chunk(ci)
            do_attn_batch(1)

            # ---- AllToAll: head-sharded -> token-sharded ----
            cc_in = dram.tile([N_CORES, FW, TSL], F32R)
            cc_out = dram.tile([N_CORES, FW, TSL], F32R)
            for p in range(N_CORES):
                nc.gpsimd.dma_start(
                    out=cc_in[p, :, :], in_=outT[:, p * TSL : (p + 1) * TSL]
                )
            nc.gpsimd.collective_compute(
                "AllToAll",
                mybir.AluOpType.bypass,
                replica_groups=[list(range(N_CORES))],
                ins=[cc_in[:].opt()],
                outs=[cc_out[:].opt()],
            )
            # reuses qT's slot (dead after attention) — Tile serializes via WAR deps
            orecv = qkpool.tile([128, N_CORES, TSL], F32R, tag="qT")
            for p in range(N_CORES):
                nc.gpsimd.dma_start(out=orecv[:, p, :], in_=cc_out[p, :, :])

            # ---- output projection for this core's token slice ----
            wo_sb = []
            for p in range(N_CORES):
                wt = wopool.tile([128, D], F32R, tag="wo")
                nc.sync.dma_start(out=wt, in_=woT[p * 128 : (p + 1) * 128, :])
                wo_sb.append(wt)
            for tt in range(TSL // 128):
                ysb = ypool.tile([128, D], F32, tag="y")
                for ec in range(D // 512):
                    yps = mmps.tile([128, 512], F32, tag="mm")
                    for p in range(N_CORES):
                        nc.tensor.matmul(
                            yps,
                            orecv[:, p, tt * 128 : (tt + 1) * 128],
                            wo_sb[p][:, ec * 512 : (ec + 1) * 512],
                            start=(p == 0),
                            stop=(p == N_CORES - 1),
                        )
                    nc.vector.tensor_copy(out=ysb[:, ec * 512 : (ec + 1) * 512], in_=yps)
                nc.sync.dma_start(out=y[tt * 128 : (tt + 1) * 128, :], in_=ysb)

    _spill_waits(nc)
    return nc


_NC_CACHE = None


def _get_nc():
    global _NC_CACHE
    if _NC_CACHE is None:
        _NC_CACHE = build_kernel()
    return _NC_CACHE


def _host_prep(x, Wq, Wk, Wv, Wo, token_positions):
    xT = np.ascontiguousarray(x.reshape(T, D).T)  # [D, T]
    WqT = np.ascontiguousarray(Wq.T) * np.float32(1.0 / math.sqrt(DK))
    WkT = np.ascontiguousarray(Wk.T)
    WvT = np.ascontiguousarray(Wv.T)
    WoT = np.ascontiguousarray(Wo.T)

    pos = token_positions.astype(np.float64)  # [S]
    i = (np.arange(FW) % DK) // 2  # pair index per row
    inv_freq = 1.0 / (10000.0 ** (2.0 * i / DK))  # [FW]
    ang = inv_freq[:, None] * pos[None, :]  # [FW, S]
    ctab = np.cos(ang).astype(np.float32)
    sgn = np.where(np.arange(FW) % 2 == 0, -1.0, 1.0)
    stab = (np.sin(ang) * sgn[:, None]).astype(np.float32)

    masks = (np.arange(896)[None, :] - 384 >= np.arange(KT)[:, None]).astype(
        np.float32
    )
    return xT, WqT, WkT, WvT, WoT, ctab, stab, masks


def kernel(x, Wq, Wk, Wv, Wo, mask, token_positions, num_heads, **run_kw):
    x = np.asarray(x)
    assert int(num_heads) == H and x.shape == (B, S, D)
    xT, WqT, WkT, WvT, WoT, ctab, stab, masks = _host_prep(
        np.asarray(x, np.float32),
        np.asarray(Wq, np.float32),
        np.asarray(Wk, np.float32),
        np.asarray(Wv, np.float32),
        np.asarray(Wo, np.float32),
        np.asarray(token_positions),
    )
    in_maps = []
    for c in range(N_CORES):
        cols = slice(c * FW, (c + 1) * FW)
        in_maps.append(
            {
                "xT": xT,
                "wq": np.ascontiguousarray(WqT[:, cols]),
                "wk": np.ascontiguousarray(WkT[:, cols]),
                "wv": np.ascontiguousarray(WvT[:, cols]),
                "woT": WoT,
                "ctab": ctab,
                "stab": stab,
                "masks": masks,
            }
        )
    nc = _get_nc()
    res = run_bass_kernel_spmd(
        nc, in_maps, core_ids=list(range(N_CORES)), **run_kw
    )
    yfull = np.concatenate([res.results[c]["y"] for c in range(N_CORES)], axis=0)
    out = yfull.reshape(B, S, D).astype(np.float32)
    kernel.last_results = res
    return out



# revision 36
# speedup vs baseline: 1.5483x; 1.5483x over previous
"""Multi-head causal attention with RoPE on 8 Trainium2 NeuronCores.

Problem: B=2, S=2048, D=1024, H=16 heads (dk=64), fp32 in/out, causal mask,
RoPE on Q/K, y = softmax(QK^T/sqrt(dk)) V projected by Wo.

Sharding: head-parallel. Core c owns 2 heads (columns c*128:(c+1)*128 of
the QKV projection output). All matmul operands are bf16 (PSUM accumulation
stays fp32): halves HBM/A2A traffic and keeps every matmul at 1 cycle/row
regardless of moving-dim size. Per core:
  1. Q^T,K^T,V^T for its heads from the full x (K-dim 1024 matmuls); V is
     flipped to token-major via PE transposes (cheaper than 32 128-row
     matmuls whose LDWEIGHTS can't be hidden),
  2. RoPE in the transposed [head_dim, token] layout (bf16 tables),
  3. causal attention with transposed scores ST[k,q]: the two heads' score
     blocks land in one 2-bank PSUM tile so a single exp covers both; score/
     exp/PV work is trimmed to the causally-valid column range of diagonal
     tiles, with only the 128x128 diagonal triangle masked on GpSimd; a ones
     column in V makes PV emit the softmax denominator l,
  4. per-batch AllToAll flips head-sharded -> token-sharded; the UNNORMALIZED
     O^T plus each head's l row ride in one [130, 256]-per-peer bf16 payload;
     softmax normalization happens post-A2A (one short reciprocal + selector
     matmul broadcast per batch), so the per-qc critical path ends at PV.
     Batch 0's collective and output projection hide under batch 1 compute,
  5. output projection for this core's 2x256 tokens; host reassembles.
Batch-1 QKV chunks are emitted interleaved with batch-0 attention qc's so
the PE always has fill work (keeps DVFS pstate high). PSUM->SBUF copy work
alternates between ACT (non-interleaved chunks) and DVE (interleaved ones)
so neither engine starves the exp stream. A tiny dummy AllToAll at kernel
start absorbs the one-time collective warmup cost. Constant loads are spread
over the scalar/gpsimd DMA queues in parallel with the first x chunk on the
sync queue.
"""

import sys

for p in ("/opt/trn_rl_repo", "/root/.axon_site/_ro/trn_rl_repo"):
    if p not in sys.path:
        sys.path.insert(0, p)

import math

import ml_dtypes
import numpy as np

import concourse.bass as bass
import concourse.tile as tile
from concourse import mybir
from concourse.bass_utils import run_bass_kernel_spmd

N_CORES = 8
B, S, D, H = 2, 2048, 1024, 16
DK = D // H          # 64
HPC = H // N_CORES   # heads per core = 2
FW = HPC * DK        # head-group width per core = 128
T = B * S            # 4096 flattened tokens
TCH = 512            # token chunk for projections
NCH = T // TCH       # 8 chunks
KT = 128             # k tile
QC = 512             # q chunk in attention
TOK = 256            # tokens per core per batch after A2A
VW = HPC * (DK + 1)  # v row width incl. ones columns = 130
CCW = FW + HPC       # A2A payload rows = 128 O rows + 2 l rows

F32 = mybir.dt.float32
F32R = mybir.dt.float32r
BF16 = mybir.dt.bfloat16

DEBUG = False  # adds qT/kTt/v/outT/linv dram dumps


def _spill_waits(nc, max_other=1):
    """walrus in this container allows 1 sync-wait per instruction; move
    excess waits onto preceding single-wait NoOps on the same engine."""
    n_new = 0
    for bb in nc.m.functions[0].blocks:
        newlist = []
        changed = False
        for inst in bb.instructions:
            si = inst.sync_info
            if si is not None and si.on_wait and len(si.on_wait) > max_other:
                waits = list(si.on_wait)
                overflow, keep = waits[:-max_other], waits[-max_other:]
                while overflow:
                    chunk, overflow = overflow[:1], overflow[1:]
                    nop = mybir.InstNoOp(
                        name=f"waitspill{n_new}-{inst.name}", ins=[], outs=[]
                    )
                    nop.engine = inst.engine
                    nop.debug = inst.debug
                    nop.sync_info = mybir.SyncInfo(on_wait=chunk, on_update=[])
                    newlist.append(nop)
                    n_new += 1
                si.on_wait = keep
                inst.sync_info = si
                changed = True
            newlist.append(inst)
        if changed:
            bb.instructions = newlist
    return n_new


def build_kernel(spill=True, detect_races=True):
    nc = bass.Bass("TRN2", num_devices=N_CORES, detect_race_conditions=detect_races)

    xT = nc.dram_tensor("xT", [D, T], BF16, kind="ExternalInput")
    wq = nc.dram_tensor("wq", [D, FW], BF16, kind="ExternalInput")  # pre-scaled
    wk = nc.dram_tensor("wk", [D, FW], BF16, kind="ExternalInput")
    wv = nc.dram_tensor("wv", [D, FW], BF16, kind="ExternalInput")
    woT = nc.dram_tensor("woT", [D, D], BF16, kind="ExternalInput")
    ctab = nc.dram_tensor("ctab", [FW, S], BF16, kind="ExternalInput")
    stab = nc.dram_tensor("stab", [FW, S], BF16, kind="ExternalInput")
    tri = nc.dram_tensor("tri", [KT, HPC * KT], BF16, kind="ExternalInput")
    idn = nc.dram_tensor("idn", [128, 128], BF16, kind="ExternalInput")
    perm = nc.dram_tensor("perm", [128, 128], BF16, kind="ExternalInput")
    sels = nc.dram_tensor("sels", [H, N_CORES * 128], BF16, kind="ExternalInput")
    y = nc.dram_tensor("y", [B * TOK, D], F32, kind="ExternalOutput")
    if DEBUG:
        dbg_q = nc.dram_tensor("dbg_q", [FW, T], BF16, kind="ExternalOutput")
        dbg_k = nc.dram_tensor("dbg_k", [FW, T], BF16, kind="ExternalOutput")
        dbg_v = nc.dram_tensor("dbg_v", [128, (T // 128) * VW], BF16, kind="ExternalOutput")
        dbg_o = nc.dram_tensor("dbg_o", [FW, T], BF16, kind="ExternalOutput")
        dbg_or = nc.dram_tensor("dbg_or", [128, B * N_CORES * TOK], BF16, kind="ExternalOutput")
        dbg_l = nc.dram_tensor("dbg_l", [B * H, TOK], BF16, kind="ExternalOutput")

    xT_r = xT.rearrange("(dt p) t -> p dt t", p=128)  # [128, 8, T]

    with tile.TileContext(nc) as tc:
        with (
            tc.tile_pool(name="const", bufs=1) as const,
            tc.tile_pool(name="xch", bufs=2) as xch,
            tc.tile_pool(name="qk", bufs=1) as qkpool,
            tc.tile_pool(name="raw", bufs=3) as rawp,
            tc.tile_pool(name="pt", bufs=3) as ptp,
            tc.tile_pool(name="lp", bufs=2) as lpool,
            tc.tile_pool(name="wo", bufs=8) as wopool,
            tc.tile_pool(name="orecv", bufs=2) as orp,
            tc.tile_pool(name="yout", bufs=2) as ypool,
            tc.tile_pool(name="psa", bufs=2, space="PSUM") as psA,
            tc.tile_pool(name="psb", bufs=2, space="PSUM") as psB,
            tc.tile_pool(name="dram", bufs=1, space="DRAM") as dram,
        ):
            # ---- collective warmup (hidden under projection phase) ----
            warm_in = dram.tile([8, 16], F32, tag="warm_in")
            warm_out = dram.tile([8, 16], F32, tag="warm_out")
            wtile = const.tile([1, 128], F32)
            nc.vector.memset(wtile, 0.0)
            nc.gpsimd.dma_start(
                out=warm_in[:, :],
                in_=wtile[:1, :128].rearrange("p (a f) -> (p a) f", a=8),
            )
            nc.gpsimd.collective_compute(
                "AllToAll",
                mybir.AluOpType.bypass,
                replica_groups=[list(range(N_CORES))],
                ins=[warm_in[:].opt()],
                outs=[warm_out[:].opt()],
            )

            # ---- constants: spread over scalar + gpsimd queues so the head
            # (first Q matmul needs only wq + x chunk 0) stays short ----
            wq_sb = const.tile([128, 8, FW], BF16)
            nc.scalar.dma_start(out=wq_sb, in_=wq.rearrange("(dt p) f -> p dt f", p=128))
            wk_sb = const.tile([128, 8, FW], BF16)
            nc.gpsimd.dma_start(out=wk_sb, in_=wk.rearrange("(dt p) f -> p dt f", p=128))
            c_sb = const.tile([FW, S], BF16)
            nc.scalar.dma_start(out=c_sb, in_=ctab[:, :])
            s_sb = const.tile([FW, S], BF16)
            nc.gpsimd.dma_start(out=s_sb, in_=stab[:, :])
            wv_sb = const.tile([128, 8, FW], BF16)
            nc.gpsimd.dma_start(out=wv_sb, in_=wv.rearrange("(dt p) f -> p dt f", p=128))
            idn_sb = const.tile([128, 128], BF16)
            nc.scalar.dma_start(out=idn_sb, in_=idn[:, :])
            perm_sb = const.tile([128, 128], BF16)
            nc.scalar.dma_start(out=perm_sb, in_=perm[:, :])
            tri_sb = const.tile([KT, HPC, KT], BF16)
            nc.scalar.dma_start(out=tri_sb, in_=tri.rearrange("k (h q) -> k h q", h=HPC))
            sel_sb = const.tile([H, N_CORES, 128], BF16)
            nc.scalar.dma_start(out=sel_sb, in_=sels.rearrange("k (p f) -> k p f", p=N_CORES))

            ones_f = const.tile([128, T // 128], F32)
            nc.vector.memset(ones_f, 1.0)

            qT = qkpool.tile([FW, T], BF16, tag="qT")
            kTt = qkpool.tile([FW, T], BF16, tag="kT")
            v_sb = qkpool.tile([128, T // 128, VW], BF16, tag="v")
            outT = qkpool.tile([FW, T], BF16, tag="outT")
            lrow = qkpool.tile([33, T], BF16, tag="lrow")  # l at rows 0 and 32
            # bake the ones columns of V
            nc.vector.tensor_copy(out=v_sb[:, :, DK], in_=ones_f)
            nc.vector.tensor_copy(out=v_sb[:, :, 2 * DK + 1], in_=ones_f)

            # ---- QKV projections + RoPE (per x chunk) ----
            # PSUM->SBUF copies ride ACT for chunks 0-3 (emitted before any
            # exp) and DVE for chunks 4-7 (which overlap batch-0 attention).
            def do_qkv_chunk(ci):
                t0 = ci * TCH
                sc = (ci % (S // TCH)) * TCH  # position within batch
                cpeng = nc.scalar if ci < 4 else nc.vector
                cpcopy = cpeng.copy if ci < 4 else nc.vector.tensor_copy
                xc = xch.tile([128, 8, TCH], BF16, tag="x", name="xc")
                nc.sync.dma_start(out=xc, in_=xT_r[:, :, t0 : t0 + TCH])

                for which, w_sb, dst, pool, tag in (
                    ("q", wq_sb, qT, psA, "a"),
                    ("k", wk_sb, kTt, psB, "b"),
                ):
                    ps = pool.tile([FW, TCH], F32, tag=tag, name=f"{which}ps")
                    for dt in range(8):
                        nc.tensor.matmul(
                            ps,
                            w_sb[:, dt, :],
                            xc[:, dt, :],
                            start=(dt == 0),
                            stop=(dt == 7),
                        )
                    raw = rawp.tile([FW, TCH], BF16, tag="raw", name="raw")
                    cpcopy(out=raw, in_=ps)
                    # pair swap across partitions via a permutation matmul
                    opool = psB if tag == "a" else psA
                    otag = "b" if tag == "a" else "a"
                    swps = opool.tile([FW, TCH], F32, tag=otag, name="swps")
                    nc.tensor.matmul(swps, perm_sb, raw, start=True, stop=True)
                    dslice = dst[:, t0 : t0 + TCH]
                    nc.vector.tensor_mul(dslice, raw, c_sb[:, sc : sc + TCH])
                    t2 = rawp.tile([FW, TCH], BF16, tag="t2", name="t2")
                    nc.vector.tensor_mul(t2, swps, s_sb[:, sc : sc + TCH])
                    nc.vector.tensor_add(dslice, dslice, t2)

                # V^T like Q/K, then PE-transpose to token-major
                vtps = psB.tile([FW, TCH], F32, tag="b", name="vtps")
                for dt in range(8):
                    nc.tensor.matmul(
                        vtps,
                        wv_sb[:, dt, :],
                        xc[:, dt, :],
                        start=(dt == 0),
                        stop=(dt == 7),
                    )
                vt_sb = rawp.tile([FW, TCH], BF16, tag="vt", name="vt_sb")
                cpcopy(out=vt_sb, in_=vtps)
                for sub in range(TCH // 128):
                    tp = psA.tile([128, 128], BF16, tag="a", name="tp")
                    nc.tensor.transpose(tp, vt_sb[:, sub * 128 : (sub + 1) * 128], idn_sb)
                    idx = t0 // 128 + sub
                    cpcopy(out=v_sb[:, idx, 0:DK], in_=tp[:, 0:DK])
                    cpcopy(out=v_sb[:, idx, DK + 1 : 2 * DK + 1], in_=tp[:, DK : 2 * DK])

            # ---- causal attention, one q-chunk of one batch ----
            # Transposed-scores flash style; both heads share one 2-bank PSUM
            # score tile so a single exp covers them. Diagonal k-tiles are
            # trimmed to their causally-valid column range [128j, 512) and
            # only the 128x128 boundary triangle is masked. Software-pipelined
            # one k-tile ahead so the PE never waits on exp. O^T and l leave
            # unnormalized (normalization happens post-A2A).
            def do_attn_qc(b, qc):
                trow = b * S + qc * QC
                nkt = 4 * (qc + 1)
                pv = psB.tile([DK + 1, HPC, QC], F32, tag="b", name="pv")

                def emit_st(kt):
                    kcol = b * S + kt * KT
                    off = KT * (kt - 4 * qc) if kt >= 4 * qc else 0
                    st = psA.tile([KT, HPC, QC], F32, tag="a", name="st")
                    for h in range(HPC):
                        fb = h * DK
                        nc.tensor.matmul(
                            st[:, h, off:],
                            kTt[fb : fb + DK, kcol : kcol + KT],
                            qT[fb : fb + DK, trow + off : trow + QC],
                            start=True,
                            stop=True,
                        )
                    pt = ptp.tile([KT, HPC, QC], BF16, tag="pt", name="pt")
                    nc.scalar.activation(
                        out=pt[:, :, off:],
                        in_=st[:, :, off:],
                        func=mybir.ActivationFunctionType.Exp,
                    )
                    if kt >= 4 * qc:
                        nc.gpsimd.tensor_mul(
                            pt[:, :, off : off + KT], pt[:, :, off : off + KT], tri_sb
                        )
                    return pt, off

                def emit_pv(kt, pt, off):
                    for h in range(HPC):
                        vcol = h * (DK + 1)
                        nc.tensor.matmul(
                            pv[:, h, off:],
                            v_sb[:, b * (S // 128) + kt, vcol : vcol + DK + 1],
                            pt[:, h, off:],
                            start=(kt == 0),
                            stop=(kt == nkt - 1),
                            skip_group_check=True,
                        )

                prev = emit_st(0)
                for kt in range(1, nkt):
                    cur = emit_st(kt)
                    emit_pv(kt - 1, *prev)
                    prev = cur
                emit_pv(nkt - 1, *prev)

                # unnormalized O^T out; l rows to partitions 0/32 of lrow
                for h in range(HPC):
                    fb = h * DK
                    nc.vector.tensor_copy(
                        out=outT[fb : fb + DK, trow : trow + QC], in_=pv[0:DK, h, :]
                    )
                    nc.vector.tensor_copy(
                        out=lrow[32 * h : 32 * h + 1, trow : trow + QC],
                        in_=pv[DK : DK + 1, h, :],
                    )

            # ---- per-batch AllToAll: head-sharded -> token-sharded.
            # Payload per peer: [130, 256] bf16 = 128 unnormalized O^T rows
            # plus the two heads' l rows. ----
            def a2a_send(b):
                cc_in = dram.tile([N_CORES, CCW, TOK], BF16, tag=f"ccin{b}", name="cc_in")
                cc_out = dram.tile(
                    [N_CORES, CCW, TOK], BF16, tag=f"ccout{b}", name="cc_out"
                )
                nc.sync.dma_start(
                    out=cc_in[:, 0:FW, :].rearrange("p f t -> f p t"),
                    in_=outT[:, b * S : (b + 1) * S].rearrange(
                        "f (p t) -> f p t", t=TOK
                    ),
                )
                for h in range(HPC):
                    for p in range(N_CORES):
                        c0 = b * S + p * TOK
                        nc.sync.dma_start(
                            out=cc_in[p, FW + h : FW + h + 1, :],
                            in_=lrow[32 * h : 32 * h + 1, c0 : c0 + TOK],
                        )
                nc.gpsimd.collective_compute(
                    "AllToAll",
                    mybir.AluOpType.bypass,
                    replica_groups=[list(range(N_CORES))],
                    ins=[cc_in[:].opt()],
                    outs=[cc_out[:].opt()],
                )
                return cc_out

            # recv + post-A2A normalization: one short reciprocal for all 16
            # head-l rows, selector matmuls broadcast 1/l over the 64-row
            # feature groups, DVE scales orecv in place.
            def a2a_recv(b, cc_out):
                orecv = orp.tile([128, N_CORES, TOK], BF16, tag="or", name="orecv")
                nc.sync.dma_start(
                    out=orecv, in_=cc_out[:, 0:FW, :].rearrange("p f t -> f p t")
                )
                lcol16 = lpool.tile([H, TOK], F32, tag="lcol", name="lcol16")
                for k in range(H):
                    nc.gpsimd.dma_start(
                        out=lcol16[k : k + 1, :],
                        in_=cc_out[k // 2, FW + k % 2 : FW + k % 2 + 1, :],
                    )
                linv16 = lpool.tile([H, TOK], BF16, tag="linv", name="linv16")
                with nc.allow_low_precision(reason="bf16 1/l for softmax scale"):
                    nc.vector.reciprocal(out=linv16, in_=lcol16)
                if DEBUG:
                    nc.sync.dma_start(out=dbg_l[b * H : (b + 1) * H, :], in_=linv16)
                ors = orp.tile([128, N_CORES, TOK], BF16, tag="ors", name="ors")
                for p in range(N_CORES):
                    lbb = psA.tile([128, TOK], F32, tag="a", name="lbb")
                    nc.tensor.matmul(lbb, sel_sb[:, p, :], linv16, start=True, stop=True)
                    nc.vector.tensor_mul(ors[:, p, :], orecv[:, p, :], lbb)
                return ors

            wo_sb = []

            def load_wo():
                for p in range(N_CORES):
                    wt = wopool.tile([128, D], BF16, tag="wo", name="wt")
                    nc.sync.dma_start(out=wt, in_=woT[p * 128 : (p + 1) * 128, :])
                    wo_sb.append(wt)

            # ---- output projection for this core's token slice of batch b ----
            def do_oproj(b, ors):
                for tt in range(TOK // 128):
                    yps = psA.tile([128, D], F32, tag="a", name="yps")
                    for ec in range(D // 512):
                        for p in range(N_CORES):
                            nc.tensor.matmul(
                                yps[:, ec * 512 : (ec + 1) * 512],
                                ors[:, p, tt * 128 : (tt + 1) * 128],
                                wo_sb[p][:, ec * 512 : (ec + 1) * 512],
                                start=(p == 0),
                                stop=(p == N_CORES - 1),
                            )
                    ysb = ypool.tile([128, D], F32, tag="y", name="ysb")
                    nc.vector.tensor_copy(out=ysb, in_=yps)
                    r0 = b * TOK + tt * 128
                    nc.sync.dma_start(out=y[r0 : r0 + 128, :], in_=ysb)

            # ---- schedule ----
            # b0 QKV -> b0 attention interleaved with b1 QKV chunks (PE fill
            # work while ACT churns exp) -> b0 A2A + out-proj hidden under b1
            # attention -> b1 A2A + out-proj tail.
            for ci in range(4):
                do_qkv_chunk(ci)
            do_attn_qc(0, 0)
            do_qkv_chunk(4)
            do_attn_qc(0, 1)
            do_qkv_chunk(5)
            load_wo()
            do_attn_qc(0, 2)
            do_qkv_chunk(6)
            do_attn_qc(0, 3)
            do_qkv_chunk(7)
            cc0 = a2a_send(0)
            do_attn_qc(1, 0)
            do_attn_qc(1, 1)
            do_attn_qc(1, 2)
            ors0 = a2a_recv(0, cc0)
            do_oproj(0, ors0)
            do_attn_qc(1, 3)
            cc1 = a2a_send(1)
            ors1 = a2a_recv(1, cc1)
            do_oproj(1, ors1)

            if DEBUG:
                nc.sync.dma_start(out=dbg_q[:, :], in_=qT)
                nc.sync.dma_start(out=dbg_k[:, :], in_=kTt)
                nc.sync.dma_start(
                    out=dbg_v.rearrange("p (i w) -> p i w", w=VW), in_=v_sb
                )
                nc.sync.dma_start(out=dbg_o[:, :], in_=outT)
                nc.sync.dma_start(
                    out=dbg_or[:, 0 : N_CORES * TOK].rearrange(
                        "p (i w) -> p i w", w=TOK
                    ),
                    in_=ors0,
                )
                nc.sync.dma_start(
                    out=dbg_or[:, N_CORES * TOK :].rearrange("p (i w) -> p i w", w=TOK),
                    in_=ors1,
                )

    if spill:
        _spill_waits(nc)
    return nc


_NC_CACHE = None


def _get_nc():
    global _NC_CACHE
    if _NC_CACHE is None:
        _NC_CACHE = build_kernel()
    return _NC_CACHE


def _host_prep(x, Wq, Wk, Wv, Wo, token_positions):
    bf = ml_dtypes.bfloat16
    xT = np.ascontiguousarray(x.reshape(T, D).T).astype(bf)  # [D, T]
    WqT = (np.ascontiguousarray(Wq.T) * np.float32(1.0 / math.sqrt(DK))).astype(bf)
    WkT = np.ascontiguousarray(Wk.T).astype(bf)
    WvT = np.ascontiguousarray(Wv.T).astype(bf)
    WoT = np.ascontiguousarray(Wo.T).astype(bf)

    pos = token_positions.astype(np.float64)  # [S]
    i = (np.arange(FW) % DK) // 2  # pair index per row
    inv_freq = 1.0 / (10000.0 ** (2.0 * i / DK))  # [FW]
    ang = inv_freq[:, None] * pos[None, :]  # [FW, S]
    ctab = np.cos(ang).astype(bf)
    sgn = np.where(np.arange(FW) % 2 == 0, -1.0, 1.0)
    stab = (np.sin(ang) * sgn[:, None]).astype(bf)

    # [KT, 2*KT]: two head copies of the diagonal-block causal triangle
    t1 = (np.arange(KT)[None, :] >= np.arange(KT)[:, None]).astype(np.float32)
    tri = np.concatenate([t1, t1], axis=1).astype(bf)
    idn = np.eye(128, dtype=np.float32).astype(bf)
    pr = np.zeros((128, 128), dtype=np.float32)
    pr[np.arange(128), np.arange(128) ^ 1] = 1.0
    pr = pr.astype(bf)
    # sels[k, p*128+f] = 1 iff k == 2p + f//64  (1/l broadcast selector)
    kk = np.arange(H)[:, None]
    pf = np.arange(N_CORES * 128)[None, :]
    sels = (kk == 2 * (pf // 128) + (pf % 128) // DK).astype(np.float32).astype(bf)
    return xT, WqT, WkT, WvT, WoT, ctab, stab, tri, idn, pr, sels


def kernel(x, Wq, Wk, Wv, Wo, mask, token_positions, num_heads, **run_kw):
    x = np.asarray(x)
    assert int(num_heads) == H and x.shape == (B, S, D)
    xT, WqT, WkT, WvT, WoT, ctab, stab, tri, idn, pr, sels = _host_prep(
        np.asarray(x, np.float32),
        np.asarray(Wq, np.float32),
        np.asarray(Wk, np.float32),
        np.asarray(Wv, np.float32),
        np.asarray(Wo, np.float32),
        np.asarray(token_positions),
    )
    in_maps = []
    for c in range(N_CORES):
        cols = slice(c * FW, (c + 1) * FW)
        in_maps.append(
            {
                "xT": xT,
                "wq": np.ascontiguousarray(WqT[:, cols]),
                "wk": np.ascontiguousarray(WkT[:, cols]),
                "wv": np.ascontiguousarray(WvT[:, cols]),
                "woT": WoT,
                "ctab": ctab,
                "stab": stab,
                "tri": tri,
                "idn": idn,
                "perm": pr,
                "sels": sels,
            }
        )
    nc = _get_nc()
    res = run_bass_kernel_spmd(
        nc, in_maps, core_ids=list(range(N_CORES)), **run_kw
    )
    # core c owns tokens [256c, 256(c+1)) of each batch
    out = np.empty((B, S, D), dtype=np.float32)
    for c in range(N_CORES):
        yc = res.results[c]["y"]
        out[0, c * TOK : (c + 1) * TOK] = yc[0:TOK]
        out[1, c * TOK : (c + 1) * TOK] = yc[TOK : 2 * TOK]
    kernel.last_results = res
    return out
